# revision 5
# baseline (speedup 1.0000x reference)
"""AssociativeEmbeddingLoss on 8 TRN2 NeuronCores, v3.

Reference, per image b (C=1, G=128 boxes):
    tl[g] = pred[b, 0, ty[g], tx[g]],  br[g] = target[b, 0, by[g], bx[g]]
    me = (tl + br) / 2
    pull_b = sum((tl-br)^2) / (2N)
    push_b = sum_{i != j} relu(1 - |me_i - me_j|) / (N*(N-1))
    out = (0.25 * sum_b pull_b, 0.25 * sum_b push_b)

Data-parallel over batch, 8 images per core (2048 scattered scalars per
core). The generic INDIRECT1D gather consumes ONE offset per output
partition row (hardware-verified), so element gathers cap at 128/instr
and 16 instructions cost ~21us of Q7 time. v3 instead uses the custom
dma_gather ucode, which iterates indices: 2048 int16 row indices pull
512B-aligned rows (idx = flat>>7 < 32768 fits int16 over the
pred||target concat buffer) in two 1024-idx instructions, landing row
i at dst[i%128, i//128]. With gather order i = m*128 + g (m = 2b+tb),
dst partition = box g, so the within-row remainder r = flat&127
(computed on DVE in the same [128(g), 16(m)] layout the match tile is
DMA-loaded into) drives a one-hot fine-select (is_equal vs an iota
const, multiply, blocked tensor_reduce) with no partition crossing.
The int16 index tile needs the ucode's 16-partition-wrapped layout
replicated to all 8 core groups: 8 tiny partition-slice DMAs build the
[16, 128] wrap and a K=16 block-identity matmul replicates it to
[128, 128].

The push term then reuses the v2 pipeline: one K=9 matmul (lhsT rows
0..7 = meT via PE transpose of the selected me columns, row 8 = -1;
rhs rows 0..7 = a static block-diagonal 0/1 pattern, row 8 = meT
flattened to [1,1024] by one SBUF->SBUF DMA) evaluated in two
[128,512] PSUM chunks pipelined through Scalar Abs(0.5x) and DVE
min(.,1)+accumulate; relu(1-|d|) = 1 - min(|d|,1) turns the push sum
into BP*N*(N-1) - minsum. Final reductions: two ones-matmuls +
activation accumulators. Each core returns [pull_partial,
push_partial]; the host sums the 8 pairs (unshard).
"""

import numpy as np

import concourse.bacc as bacc
import concourse.mybir as mybir
import concourse.tile as tile
from concourse.bass_utils import run_bass_kernel_spmd

B, C, H, W = 64, 1, 512, 512
G = 128                 # boxes per image; N = G*C = 128
N = G * C
NCORES = 8
BP = B // NCORES        # images per core
NPIX = BP * H * W
M = 2 * BP              # gather chunks: m = 2b + tb
NROWS = 2 * NPIX // 128
PULL_W, PUSH_W = 0.25, 0.25

F32 = mybir.dt.float32
I32 = mybir.dt.int32
I16 = mybir.dt.int16
AF = mybir.ActivationFunctionType
ALU = mybir.AluOpType
AX = mybir.AxisListType

C_PULL = PULL_W / (2.0 * N)
C_PUSH = PUSH_W / (N * (N - 1))

# cbig layout: [:, 0:2048] iota (col % 128); [:, 2048:2064] basepat;
# [:, 2064] ones; [:, 2065:2193] identity
CB_IOTA = 0
CB_BASE = 2048
CB_ONES = 2064
CB_ID = 2065
CB_W = 2193
# c2 layout: rows 0..7 cols 0:1024 block-diag deltas; row 8 cols
# 1024:1152 = -1; rows 0..15 cols 1152:1280 = Lrep (p%16 == k)
C2_W = 1280


def _build_nc():
    nc = bacc.Bacc(
        "TRN2",
        target_bir_lowering=False,
        debug=False,
        enable_asserts=False,
        num_devices=NCORES,
    )
    data = nc.dram_tensor("data", [NROWS, 128], F32, kind="ExternalInput")
    match = nc.dram_tensor("match", [BP * G * 4, 1], F32, kind="ExternalInput")
    c2d = nc.dram_tensor("c2", [16, C2_W], F32, kind="ExternalInput")
    cbd = nc.dram_tensor("cbig", [G, CB_W], F32, kind="ExternalInput")
    out = nc.dram_tensor("out", [1, 2], F32, kind="ExternalOutput")

    with tile.TileContext(nc) as tc:
        _kernel_body(nc, tc, data, match, c2d, cbd, out)
    nc.compile()
    return nc


def _kernel_body(nc, tc, data, match, c2d, cbd, out):
    with (
        tc.tile_pool(name="sb", bufs=1) as sb,
        tc.tile_pool(name="ps", bufs=1, space="PSUM") as ps,
    ):
        # ---- loads ----
        mg = sb.tile([G, 4 * BP], F32, tag="mg")
        msrc = match.ap()
        msrc.ap = mybir.VecI64Pair([[4, G], [4 * G, BP], [1, 4]])
        nc.sync.dma_start(out=mg[:], in_=msrc)

        c2 = sb.tile([16, C2_W], F32, tag="c2")
        nc.scalar.dma_start(out=c2[:], in_=c2d.ap())
        cb = sb.tile([G, CB_W], F32, tag="cb")
        nc.sync.dma_start(out=cb[:], in_=cbd.ap())

        iota = cb[:, CB_IOTA : CB_IOTA + 2048]
        basepat = cb[:, CB_BASE : CB_BASE + M]
        ones = cb[:, CB_ONES : CB_ONES + 1]
        ident = cb[:, CB_ID : CB_ID + G]
        rhs = c2[0 : BP + 1, 0:1024]
        lhsT = c2[0 : BP + 1, 1024:1152]
        merow = c2[BP : BP + 1, 0:1024]
        meT = c2[0:BP, 1024:1152]
        lrep = c2[0:16, 1152:1280]

        # preload the activation table while DMAs are in flight
        scrd = sb.tile([1, 1], F32, tag="scrd")
        nc.scalar.activation(out=scrd[:], in_=c2[0:1, 0:1], func=AF.Abs)

        # ---- flat offsets in the native [128(g), 16(m=2b+tb)] layout ----
        mgv = mg[:].rearrange("g (m yx) -> g m yx", m=M, yx=2)
        ft = sb.tile([G, M], F32, tag="ft")
        nc.vector.tensor_scalar(
            out=ft[:], in0=mgv[:, :, 0], scalar1=float(W), scalar2=None,
            op0=ALU.mult,
        )
        nc.vector.tensor_tensor(out=ft[:], in0=ft[:], in1=mgv[:, :, 1], op=ALU.add)
        nc.vector.tensor_tensor(out=ft[:], in0=ft[:], in1=basepat, op=ALU.add)
        fti = sb.tile([G, M], I32, tag="fti")
        nc.vector.tensor_copy(out=fti[:], in_=ft[:])
        idxi = sb.tile([G, M], I32, tag="idxi")
        nc.vector.tensor_scalar(
            out=idxi[:], in0=fti[:], scalar1=7, scalar2=None,
            op0=ALU.logical_shift_right,
        )
        idxf = sb.tile([G, M], F32, tag="idxf")
        nc.vector.tensor_copy(out=idxf[:], in_=idxi[:])
        ri = sb.tile([G, M], I32, tag="ri")
        nc.vector.tensor_scalar(
            out=ri[:], in0=fti[:], scalar1=127, scalar2=None,
            op0=ALU.bitwise_and,
        )
        rf = sb.tile([G, M], F32, tag="rf")
        nc.vector.tensor_copy(out=rf[:], in_=ri[:])

        # ---- wrap layout for the gather ucode + replicate to 8 groups ----
        # wrap[p16, m*8+gh] = idx[gh*16+p16, m]
        wrapf = sb.tile([16, G], F32, tag="wrapf")
        wrv = wrapf[:].rearrange("p (m gh) -> p m gh", m=M, gh=8)
        for gh in range(8):
            eng = nc.sync if gh % 2 == 0 else nc.scalar
            eng.dma_start(out=wrv[:, :, gh : gh + 1],
                          in_=idxf[gh * 16 : (gh + 1) * 16, :])
        repP = ps.tile([G, G], F32, tag="repP")
        nc.tensor.matmul(out=repP[:], lhsT=lrep, rhs=wrapf[:],
                         start=True, stop=True)
        it16 = sb.tile([G, G], I16, tag="it16")
        nc.vector.tensor_copy(out=it16[:], in_=repP[:])

        # ---- gather: row i = m*128+g lands at dst[g, m*128:...] ----
        dst = sb.tile([G, M * 128], F32, tag="dst")
        dstv = dst[:].rearrange("p (c e) -> p c e", c=M, e=128)
        for h in range(2):
            nc.gpsimd.dma_gather(
                out_ap=dstv[:, h * 8 : (h + 1) * 8, :],
                in_ap=data.ap(),
                idxs_ap=it16[:, h * 64 : (h + 1) * 64],
                num_idxs=1024, num_idxs_reg=1024, elem_size=128,
            )

        # ---- fine select: v[g, m] = dst[g, m*128 + r[g, m]] ----
        sel = sb.tile([G, M * 128], F32, tag="sel")
        selv = sel[:].rearrange("p (c e) -> p c e", c=M, e=128)
        iov = iota.rearrange("p (c e) -> p c e", c=M, e=128)
        nc.vector.tensor_tensor(
            out=selv[:], in0=iov[:], in1=rf[:].to_broadcast([G, M, 128]),
            op=ALU.is_equal,
        )
        vv = sb.tile([G, M], F32, tag="vv")
        for h in range(2):
            cs = slice(h * 1024, (h + 1) * 1024)
            nc.vector.tensor_tensor(
                out=sel[:, cs], in0=sel[:, cs], in1=dst[:, cs], op=ALU.mult,
            )
            nc.vector.tensor_reduce(
                out=vv[:, h * 8 : (h + 1) * 8],
                in_=selv[:, h * 8 : (h + 1) * 8, :], axis=AX.X, op=ALU.add,
            )

        # ---- me / dsub / sq in [128(g), 8(b)] columns ----
        vvv = vv[:].rearrange("g (b t) -> g b t", b=BP, t=2)
        me = sb.tile([G, BP], F32, tag="me")
        nc.vector.tensor_tensor(out=me[:], in0=vvv[:, :, 0], in1=vvv[:, :, 1], op=ALU.add)
        dsub = sb.tile([G, BP], F32, tag="dsub")
        nc.vector.tensor_tensor(out=dsub[:], in0=vvv[:, :, 0], in1=vvv[:, :, 1], op=ALU.subtract)
        sq = sb.tile([G, BP], F32, tag="sq")
        nc.vector.tensor_tensor(out=sq[:], in0=dsub[:], in1=dsub[:], op=ALU.mult)

        # ---- push: meT -> lhsT rows 0..7; merow via flatten DMA ----
        meTp = ps.tile([BP, G], F32, tag="meTp")
        nc.tensor.transpose(out=meTp[:], in_=me[:], identity=ident)
        nc.vector.tensor_copy(out=meT, in_=meTp[:])
        nc.sync.dma_start(out=merow, in_=meT)

        mcol = sb.tile([G, 2], F32, tag="mcol")
        for h in range(2):
            cs = slice(h * 512, (h + 1) * 512)
            Rp = ps.tile([G, 512], F32, tag=f"Rp{h}")
            nc.tensor.matmul(out=Rp[:], lhsT=lhsT, rhs=rhs[:, cs],
                             start=True, stop=True)
            ad = sb.tile([G, 512], F32, tag=f"ad{h}")
            nc.scalar.activation(out=ad[:], in_=Rp[:], func=AF.Abs, scale=0.5)
            nc.vector.tensor_scalar(
                out=ad[:], in0=ad[:], scalar1=1.0, scalar2=0.0,
                op0=ALU.min, op1=ALU.add, accum_out=mcol[:, h : h + 1],
            )

        # ---- final reductions ----
        pr = ps.tile([1, BP], F32, tag="pr")
        nc.tensor.matmul(out=pr[:], lhsT=ones, rhs=sq[:], start=True, stop=True)
        pp = ps.tile([1, 2], F32, tag="pp")
        nc.tensor.matmul(out=pp[:], lhsT=ones, rhs=mcol[:], start=True, stop=True)
        scr = sb.tile([1, BP + 2], F32, tag="scr")
        res = sb.tile([1, 2], F32, tag="res")
        nc.scalar.activation(out=scr[0:1, 0:BP], in_=pr[:], func=AF.Copy,
                             scale=C_PULL, accum_out=res[0:1, 0:1])
        nc.scalar.activation(out=scr[0:1, BP : BP + 2], in_=pp[:], func=AF.Copy,
                             scale=-C_PUSH,
                             bias=float(BP * N * (N - 1)) * C_PUSH / 2.0,
                             accum_out=res[0:1, 1:2])
        nc.sync.dma_start(out=out.ap(), in_=res[:])


_NC_CACHE = None


def _get_nc():
    global _NC_CACHE
    if _NC_CACHE is None:
        _NC_CACHE = _build_nc()
    return _NC_CACHE


def _consts():
    cb = np.zeros((G, CB_W), dtype=np.float32)
    cb[:, CB_IOTA : CB_IOTA + 2048] = np.tile(
        np.arange(128, dtype=np.float32), M)[None, :]
    for q in range(M):
        cb[:, CB_BASE + q] = (q // 2) * H * W + (q % 2) * NPIX
    cb[:, CB_ONES] = 1.0
    cb[:, CB_ID : CB_ID + G] = np.eye(G, dtype=np.float32)

    c2 = np.zeros((16, C2_W), dtype=np.float32)
    for b in range(BP):
        c2[b, 128 * b : 128 * (b + 1)] = 1.0
    c2[BP, 1024:1152] = -1.0
    for k in range(16):
        c2[k, 1152 + k : 1280 : 16] = 1.0
    return cb, c2


def make_in_maps(pred, target, match):
    pred = np.asarray(pred, dtype=np.float32).reshape(B, H * W)
    target = np.asarray(target, dtype=np.float32).reshape(B, H * W)
    match = np.asarray(match)
    cb, c2 = _consts()
    in_maps = []
    for k in range(NCORES):
        sl = slice(k * BP, (k + 1) * BP)
        data = np.concatenate(
            [pred[sl].reshape(-1), target[sl].reshape(-1)]
        ).reshape(NROWS, 128)
        in_maps.append({
            "data": data,
            "match": np.ascontiguousarray(match[sl]).astype(np.float32).reshape(BP * G * 4, 1),
            "c2": c2,
            "cbig": cb,
        })
    return in_maps


def kernel(pred, target, match, _trace=False):
    nc = _get_nc()
    in_maps = make_in_maps(pred, target, match)
    res = run_bass_kernel_spmd(nc, in_maps, core_ids=list(range(NCORES)), trace=_trace)
    total = np.zeros((1, 2), dtype=np.float64)
    for r in res.results:
        total += r["out"].astype(np.float64)
    out = (np.float32(total[0, 0]), np.float32(total[0, 1]))
    if _trace:
        return out, res
    return out


# revision 6
# speedup vs baseline: 1.5651x; 1.5651x over previous
"""AssociativeEmbeddingLoss on 8 TRN2 NeuronCores, v4.

Reference, per image b (C=1, G=128 boxes):
    tl[g] = pred[b, 0, ty[g], tx[g]],  br[g] = target[b, 0, by[g], bx[g]]
    me = (tl + br) / 2
    pull_b = sum((tl-br)^2) / (2N)
    push_b = sum_{i != j} relu(1 - |me_i - me_j|) / (N*(N-1))
    out = (0.25 * sum_b pull_b, 0.25 * sum_b push_b)

Data-parallel over batch, 8 images per core (2048 scattered scalars).
SWDGE descriptor generation runs at ~10.3ns/descriptor regardless of
how the gather is expressed (16x128 INDIRECT1D and 2x1024 DMAGatherAnt
both measure ~21us of Q7 time), so ~21us/core is the hard floor and
everything else must hide behind it:

- match is DMA-loaded straight into the [128(g), 16(2b+tb)] layout via
  a 3-dim access pattern, so the 2048 flat offsets cost just 4 DVE ops
  (y*W, +x, +base(b,tb), int convert) - no PE transposes, and all 16
  gather columns are ready before the first gather issues.
- pred||target are concatenated host-side into one DRAM buffer (base
  for br columns includes +NPIX) so all 16 gathers read one tensor.
- the 16 [128,1] indirect gathers stream back-to-back on the GpSimd
  queue with no interleaved dependencies; per-image push compute
  (PE transpose -> K=1 ones x me_row matmul -> Scalar Abs(0.5x - me_i)
  -> DVE min(.,1)+accumulate, with the DVE min lagging one image to
  avoid queue stalls) is pipelined two gathers (~2.7us) behind.
- pull reuses the gathered columns: dsub/sq as two bulk strided DVE
  ops into the same [128, 16] tile as the 8 min-accumulator columns,
  reduced by a single ones-matmul + two activation accumulators.
Each core returns [pull_partial, push_partial]; the host sums the 8
pairs (unshard).
"""

import numpy as np

import concourse.bacc as bacc
import concourse.mybir as mybir
import concourse.tile as tile
from concourse.bass import IndirectOffsetOnAxis
from concourse.bass_utils import run_bass_kernel_spmd

B, C, H, W = 64, 1, 512, 512
G = 128                 # boxes per image; N = G*C = 128
N = G * C
NCORES = 8
BP = B // NCORES        # images per core
NPIX = BP * H * W
M = 2 * BP              # gather columns: m = 2b + tb
PULL_W, PUSH_W = 0.25, 0.25

F32 = mybir.dt.float32
I32 = mybir.dt.int32
AF = mybir.ActivationFunctionType
ALU = mybir.AluOpType

C_PULL = PULL_W / (2.0 * N)
C_PUSH = PUSH_W / (N * (N - 1))

# cbig layout: [:, 0:128] identity; [:, 128] ones col;
# [0, 129:257] ones row; [:, 257:273] basepat
CB_ID = 0
CB_ONES = 128
CB_ONESROW = 129
CB_BASE = 257
CB_W = 273


def _build_nc():
    nc = bacc.Bacc(
        "TRN2",
        target_bir_lowering=False,
        debug=False,
        enable_asserts=False,
        num_devices=NCORES,
    )
    data = nc.dram_tensor("data", [2 * NPIX, 1], F32, kind="ExternalInput")
    match = nc.dram_tensor("match", [BP * G * 4, 1], F32, kind="ExternalInput")
    cbd = nc.dram_tensor("cbig", [G, CB_W], F32, kind="ExternalInput")
    out = nc.dram_tensor("out", [1, 2], F32, kind="ExternalOutput")

    with tile.TileContext(nc) as tc:
        _kernel_body(nc, tc, data, match, cbd, out)
    nc.compile()
    return nc


def _kernel_body(nc, tc, data, match, cbd, out):
    with (
        tc.tile_pool(name="sb", bufs=1) as sb,
        tc.tile_pool(name="ps", bufs=1, space="PSUM") as ps,
        tc.tile_pool(name="psr", bufs=2, space="PSUM") as psr,
    ):
        # ---- loads ----
        mg = sb.tile([G, 4 * BP], F32, tag="mg")
        msrc = match.ap()
        msrc.ap = mybir.VecI64Pair([[4, G], [4 * G, BP], [1, 4]])
        nc.sync.dma_start(out=mg[:], in_=msrc)
        cb = sb.tile([G, CB_W], F32, tag="cb")
        nc.sync.dma_start(out=cb[:], in_=cbd.ap())
        ident = cb[:, CB_ID : CB_ID + G]
        ones = cb[:, CB_ONES : CB_ONES + 1]
        ones_row = cb[0:1, CB_ONESROW : CB_ONESROW + G]
        basepat = cb[:, CB_BASE : CB_BASE + M]

        # preload the activation table while DMAs are in flight
        scrd = sb.tile([1, 1], F32, tag="scrd")
        nc.scalar.activation(out=scrd[:], in_=cb[0:1, CB_ONES : CB_ONES + 1],
                             func=AF.Abs)

        # ---- flat offsets, native [128(g), 16(m)] layout ----
        mgv = mg[:].rearrange("g (m yx) -> g m yx", m=M, yx=2)
        ft = sb.tile([G, M], F32, tag="ft")
        nc.vector.tensor_scalar(
            out=ft[:], in0=mgv[:, :, 0], scalar1=float(W), scalar2=None,
            op0=ALU.mult,
        )
        nc.vector.tensor_tensor(out=ft[:], in0=ft[:], in1=mgv[:, :, 1], op=ALU.add)
        nc.vector.tensor_tensor(out=ft[:], in0=ft[:], in1=basepat, op=ALU.add)
        fti = sb.tile([G, M], I32, tag="fti")
        nc.vector.tensor_copy(out=fti[:], in_=ft[:])

        # ---- 16 gathers streaming on gpsimd; per-image push pipelined ----
        dcol = sb.tile([G, M], F32, tag="dcol")
        for m in range(M):
            nc.gpsimd.indirect_dma_start(
                out=dcol[:, m : m + 1], out_offset=None, in_=data.ap(),
                in_offset=IndirectOffsetOnAxis(ap=fti[:, m : m + 1], axis=0),
            )

        dv = dcol[:].rearrange("g (b t) -> g b t", b=BP, t=2)
        me = sb.tile([G, BP], F32, tag="me")
        negme = sb.tile([G, BP], F32, tag="negme")
        fin = sb.tile([G, 2 * BP], F32, tag="fin")   # cols 0:8 sq, 8:16 min

        def push_image(b):
            bs = slice(b, b + 1)
            nc.vector.tensor_tensor(out=me[:, bs], in0=dv[:, b, 0:1],
                                    in1=dv[:, b, 1:2], op=ALU.add)
            nc.vector.tensor_scalar(out=negme[:, bs], in0=me[:, bs],
                                    scalar1=-0.5, scalar2=None, op0=ALU.mult)
            rowp = psr.tile([1, G], F32, tag="rowp")
            nc.tensor.transpose(out=rowp[:], in_=me[:, bs], identity=ident)
            merow = sb.tile([1, G], F32, tag=f"merow{b % 2}")
            nc.vector.tensor_copy(out=merow[:], in_=rowp[:])
            Rp = psr.tile([G, G], F32, tag="Rp")
            nc.tensor.matmul(out=Rp[:], lhsT=ones_row, rhs=merow[:],
                             start=True, stop=True)
            ad = sb.tile([G, G], F32, tag=f"ad{b % 2}")
            nc.scalar.activation(out=ad[:], in_=Rp[:], func=AF.Abs,
                                 bias=negme[:, bs], scale=0.5)
            return ad

        ads = [None, None]
        for b in range(BP):
            # lag the DVE min by one image so the vector queue never
            # stalls waiting on this image's ABS
            if b >= 1:
                pb = b - 1
                nc.vector.tensor_scalar(
                    out=ads[pb % 2][:], in0=ads[pb % 2][:], scalar1=1.0,
                    scalar2=0.0, op0=ALU.min, op1=ALU.add,
                    accum_out=fin[:, BP + pb : BP + pb + 1],
                )
            ads[b % 2] = push_image(b)

        # pull: bulk dsub/sq while the last ABS runs
        dsub = sb.tile([G, BP], F32, tag="dsub")
        nc.vector.tensor_tensor(out=dsub[:], in0=dv[:, :, 0], in1=dv[:, :, 1],
                                op=ALU.subtract)
        nc.vector.tensor_tensor(out=fin[:, 0:BP], in0=dsub[:], in1=dsub[:],
                                op=ALU.mult)
        nc.vector.tensor_scalar(
            out=ads[(BP - 1) % 2][:], in0=ads[(BP - 1) % 2][:], scalar1=1.0,
            scalar2=0.0, op0=ALU.min, op1=ALU.add,
            accum_out=fin[:, 2 * BP - 1 : 2 * BP],
        )

        # ---- final reduction: one ones-matmul + two accum activations ----
        pg = ps.tile([1, 2 * BP], F32, tag="pg")
        nc.tensor.matmul(out=pg[:], lhsT=ones, rhs=fin[:], start=True, stop=True)
        scr = sb.tile([1, 2 * BP], F32, tag="scr")
        res = sb.tile([1, 2], F32, tag="res")
        nc.scalar.activation(out=scr[0:1, 0:BP], in_=pg[0:1, 0:BP], func=AF.Copy,
                             scale=C_PULL, accum_out=res[0:1, 0:1])
        nc.scalar.activation(out=scr[0:1, BP : 2 * BP], in_=pg[0:1, BP : 2 * BP],
                             func=AF.Copy, scale=-C_PUSH,
                             bias=float(BP * N * (N - 1)) * C_PUSH / BP,
                             accum_out=res[0:1, 1:2])
        nc.sync.dma_start(out=out.ap(), in_=res[:])


_NC_CACHE = None


def _get_nc():
    global _NC_CACHE
    if _NC_CACHE is None:
        _NC_CACHE = _build_nc()
    return _NC_CACHE


def _consts():
    cb = np.zeros((G, CB_W), dtype=np.float32)
    cb[:, CB_ID : CB_ID + G] = np.eye(G, dtype=np.float32)
    cb[:, CB_ONES] = 1.0
    cb[0, CB_ONESROW : CB_ONESROW + G] = 1.0
    for q in range(M):
        cb[:, CB_BASE + q] = (q // 2) * H * W + (q % 2) * NPIX
    return cb


def make_in_maps(pred, target, match):
    pred = np.asarray(pred, dtype=np.float32).reshape(B, H * W)
    target = np.asarray(target, dtype=np.float32).reshape(B, H * W)
    match = np.asarray(match)
    cb = _consts()
    in_maps = []
    for k in range(NCORES):
        sl = slice(k * BP, (k + 1) * BP)
        data = np.concatenate(
            [pred[sl].reshape(-1), target[sl].reshape(-1)]
        ).reshape(2 * NPIX, 1)
        in_maps.append({
            "data": data,
            "match": np.ascontiguousarray(match[sl]).astype(np.float32).reshape(BP * G * 4, 1),
            "cbig": cb,
        })
    return in_maps


def kernel(pred, target, match, _trace=False):
    nc = _get_nc()
    in_maps = make_in_maps(pred, target, match)
    res = run_bass_kernel_spmd(nc, in_maps, core_ids=list(range(NCORES)), trace=_trace)
    total = np.zeros((1, 2), dtype=np.float64)
    for r in res.results:
        total += r["out"].astype(np.float64)
    out = (np.float32(total[0, 0]), np.float32(total[0, 1]))
    if _trace:
        return out, res
    return out


# revision 7
# speedup vs baseline: 1.5884x; 1.0149x over previous
"""AssociativeEmbeddingLoss on 8 TRN2 NeuronCores, v4.

Reference, per image b (C=1, G=128 boxes):
    tl[g] = pred[b, 0, ty[g], tx[g]],  br[g] = target[b, 0, by[g], bx[g]]
    me = (tl + br) / 2
    pull_b = sum((tl-br)^2) / (2N)
    push_b = sum_{i != j} relu(1 - |me_i - me_j|) / (N*(N-1))
    out = (0.25 * sum_b pull_b, 0.25 * sum_b push_b)

Data-parallel over batch, 8 images per core (2048 scattered scalars).
SWDGE descriptor generation runs at ~10.3ns/descriptor regardless of
how the gather is expressed (16x128 INDIRECT1D and 2x1024 DMAGatherAnt
both measure ~21us of Q7 time), so ~21us/core is the hard floor and
everything else must hide behind it:

- match is DMA-loaded straight into the [128(g), 16(2b+tb)] layout via
  a 3-dim access pattern, so the 2048 flat offsets cost just 4 DVE ops
  (y*W, +x, +base(b,tb), int convert) - no PE transposes, and all 16
  gather columns are ready before the first gather issues.
- pred||target are concatenated host-side into one DRAM buffer (base
  for br columns includes +NPIX) so all 16 gathers read one tensor.
- the 16 [128,1] indirect gathers stream back-to-back on the GpSimd
  queue with no interleaved dependencies; per-image push compute
  (PE transpose -> K=1 ones x me_row matmul -> Scalar Abs(0.5x - me_i)
  -> DVE min(.,1)+accumulate, with the DVE min lagging one image to
  avoid queue stalls) is pipelined two gathers (~2.7us) behind.
- pull reuses the gathered columns: dsub/sq as two bulk strided DVE
  ops into the same [128, 16] tile as the 8 min-accumulator columns,
  reduced by a single ones-matmul + two activation accumulators.
Each core returns [pull_partial, push_partial]; the host sums the 8
pairs (unshard).
"""

import numpy as np

import concourse.bacc as bacc
import concourse.mybir as mybir
import concourse.tile as tile
from concourse.bass import IndirectOffsetOnAxis
from concourse.bass_utils import run_bass_kernel_spmd

B, C, H, W = 64, 1, 512, 512
G = 128                 # boxes per image; N = G*C = 128
N = G * C
NCORES = 8
BP = B // NCORES        # images per core
NPIX = BP * H * W
M = 2 * BP              # gather columns: m = 2b + tb
PULL_W, PUSH_W = 0.25, 0.25

F32 = mybir.dt.float32
I32 = mybir.dt.int32
AF = mybir.ActivationFunctionType
ALU = mybir.AluOpType

C_PULL = PULL_W / (2.0 * N)
C_PUSH = PUSH_W / (N * (N - 1))

# cbig layout: [:, 0:128] identity; [:, 128] ones col;
# [0, 129:257] ones row; [:, 257:273] basepat
CB_ID = 0
CB_ONES = 128
CB_ONESROW = 129
CB_BASE = 257
CB_W = 273


def _build_nc():
    nc = bacc.Bacc(
        "TRN2",
        target_bir_lowering=False,
        debug=False,
        enable_asserts=False,
        num_devices=NCORES,
    )
    data = nc.dram_tensor("data", [2 * NPIX, 1], F32, kind="ExternalInput")
    match = nc.dram_tensor("match", [BP * G * 4, 1], F32, kind="ExternalInput")
    cbd = nc.dram_tensor("cbig", [G, CB_W], F32, kind="ExternalInput")
    out = nc.dram_tensor("out", [1, 2], F32, kind="ExternalOutput")

    with tile.TileContext(nc) as tc:
        _kernel_body(nc, tc, data, match, cbd, out)
    nc.compile()
    return nc


def _kernel_body(nc, tc, data, match, cbd, out):
    with (
        tc.tile_pool(name="sb", bufs=1) as sb,
        tc.tile_pool(name="ps", bufs=1, space="PSUM") as ps,
        tc.tile_pool(name="psr", bufs=2, space="PSUM") as psr,
    ):
        # ---- loads ----
        mg = sb.tile([G, 4 * BP], F32, tag="mg")
        msrc = match.ap()
        msrc.ap = mybir.VecI64Pair([[4, G], [4 * G, BP], [1, 4]])
        nc.sync.dma_start(out=mg[:], in_=msrc)
        cb = sb.tile([G, CB_W], F32, tag="cb")
        nc.sync.dma_start(out=cb[:], in_=cbd.ap())
        ident = cb[:, CB_ID : CB_ID + G]
        ones = cb[:, CB_ONES : CB_ONES + 1]
        ones_row = cb[0:1, CB_ONESROW : CB_ONESROW + G]
        basepat = cb[:, CB_BASE : CB_BASE + M]

        # preload the activation table while DMAs are in flight
        scrd = sb.tile([1, 1], F32, tag="scrd")
        nc.scalar.activation(out=scrd[:], in_=cb[0:1, CB_ONES : CB_ONES + 1],
                             func=AF.Abs)

        # ---- flat offsets, native [128(g), 16(m)] layout ----
        # compute columns 0:2 first so gather 0 can issue ~0.6us earlier
        mgv = mg[:].rearrange("g (m yx) -> g m yx", m=M, yx=2)
        ft = sb.tile([G, M], F32, tag="ft")
        fti = sb.tile([G, M], I32, tag="fti")
        for cs in (slice(0, 2), slice(2, M)):
            nc.vector.tensor_scalar(
                out=ft[:, cs], in0=mgv[:, cs, 0], scalar1=float(W), scalar2=None,
                op0=ALU.mult,
            )
            nc.vector.tensor_tensor(out=ft[:, cs], in0=ft[:, cs],
                                    in1=mgv[:, cs, 1], op=ALU.add)
            nc.vector.tensor_tensor(out=ft[:, cs], in0=ft[:, cs],
                                    in1=basepat[:, cs], op=ALU.add)
            nc.vector.tensor_copy(out=fti[:, cs], in_=ft[:, cs])

        # ---- 16 gathers streaming on gpsimd; per-image push pipelined ----
        dcol = sb.tile([G, M], F32, tag="dcol")
        for m in range(M):
            nc.gpsimd.indirect_dma_start(
                out=dcol[:, m : m + 1], out_offset=None, in_=data.ap(),
                in_offset=IndirectOffsetOnAxis(ap=fti[:, m : m + 1], axis=0),
            )

        dv = dcol[:].rearrange("g (b t) -> g b t", b=BP, t=2)
        me = sb.tile([G, BP], F32, tag="me")
        negme = sb.tile([G, BP], F32, tag="negme")
        fin = sb.tile([G, 2 * BP], F32, tag="fin")   # cols 0:8 sq, 8:16 min

        def push_image(b):
            bs = slice(b, b + 1)
            nc.vector.tensor_tensor(out=me[:, bs], in0=dv[:, b, 0:1],
                                    in1=dv[:, b, 1:2], op=ALU.add)
            nc.vector.tensor_scalar(out=negme[:, bs], in0=me[:, bs],
                                    scalar1=-0.5, scalar2=None, op0=ALU.mult)
            rowp = psr.tile([1, G], F32, tag="rowp")
            nc.tensor.transpose(out=rowp[:], in_=me[:, bs], identity=ident)
            merow = sb.tile([1, G], F32, tag=f"merow{b % 2}")
            nc.vector.tensor_copy(out=merow[:], in_=rowp[:])
            Rp = psr.tile([G, G], F32, tag="Rp")
            nc.tensor.matmul(out=Rp[:], lhsT=ones_row, rhs=merow[:],
                             start=True, stop=True)
            ad = sb.tile([G, G], F32, tag=f"ad{b % 2}")
            nc.scalar.activation(out=ad[:], in_=Rp[:], func=AF.Abs,
                                 bias=negme[:, bs], scale=0.5)
            return ad

        ads = [None, None]
        for b in range(BP):
            # lag the DVE min by one image so the vector queue never
            # stalls waiting on this image's ABS
            if b >= 1:
                pb = b - 1
                nc.vector.tensor_scalar(
                    out=ads[pb % 2][:], in0=ads[pb % 2][:], scalar1=1.0,
                    scalar2=0.0, op0=ALU.min, op1=ALU.add,
                    accum_out=fin[:, BP + pb : BP + pb + 1],
                )
            ads[b % 2] = push_image(b)

        # pull: bulk dsub/sq while the last ABS runs
        dsub = sb.tile([G, BP], F32, tag="dsub")
        nc.vector.tensor_tensor(out=dsub[:], in0=dv[:, :, 0], in1=dv[:, :, 1],
                                op=ALU.subtract)
        nc.vector.tensor_tensor(out=fin[:, 0:BP], in0=dsub[:], in1=dsub[:],
                                op=ALU.mult)
        nc.vector.tensor_scalar(
            out=ads[(BP - 1) % 2][:], in0=ads[(BP - 1) % 2][:], scalar1=1.0,
            scalar2=0.0, op0=ALU.min, op1=ALU.add,
            accum_out=fin[:, 2 * BP - 1 : 2 * BP],
        )

        # ---- final reduction: one ones-matmul + two accum activations ----
        pg = ps.tile([1, 2 * BP], F32, tag="pg")
        nc.tensor.matmul(out=pg[:], lhsT=ones, rhs=fin[:], start=True, stop=True)
        scr = sb.tile([1, 2 * BP], F32, tag="scr")
        res = sb.tile([1, 2], F32, tag="res")
        nc.scalar.activation(out=scr[0:1, 0:BP], in_=pg[0:1, 0:BP], func=AF.Copy,
                             scale=C_PULL, accum_out=res[0:1, 0:1])
        nc.scalar.activation(out=scr[0:1, BP : 2 * BP], in_=pg[0:1, BP : 2 * BP],
                             func=AF.Copy, scale=-C_PUSH,
                             bias=float(BP * N * (N - 1)) * C_PUSH / BP,
                             accum_out=res[0:1, 1:2])
        nc.sync.dma_start(out=out.ap(), in_=res[:])


_NC_CACHE = None


def _get_nc():
    global _NC_CACHE
    if _NC_CACHE is None:
        _NC_CACHE = _build_nc()
    return _NC_CACHE


def _consts():
    cb = np.zeros((G, CB_W), dtype=np.float32)
    cb[:, CB_ID : CB_ID + G] = np.eye(G, dtype=np.float32)
    cb[:, CB_ONES] = 1.0
    cb[0, CB_ONESROW : CB_ONESROW + G] = 1.0
    for q in range(M):
        cb[:, CB_BASE + q] = (q // 2) * H * W + (q % 2) * NPIX
    return cb


def make_in_maps(pred, target, match):
    pred = np.asarray(pred, dtype=np.float32).reshape(B, H * W)
    target = np.asarray(target, dtype=np.float32).reshape(B, H * W)
    match = np.asarray(match)
    cb = _consts()
    in_maps = []
    for k in range(NCORES):
        sl = slice(k * BP, (k + 1) * BP)
        data = np.concatenate(
            [pred[sl].reshape(-1), target[sl].reshape(-1)]
        ).reshape(2 * NPIX, 1)
        in_maps.append({
            "data": data,
            "match": np.ascontiguousarray(match[sl]).astype(np.float32).reshape(BP * G * 4, 1),
            "cbig": cb,
        })
    return in_maps


def kernel(pred, target, match, _trace=False):
    nc = _get_nc()
    in_maps = make_in_maps(pred, target, match)
    res = run_bass_kernel_spmd(nc, in_maps, core_ids=list(range(NCORES)), trace=_trace)
    total = np.zeros((1, 2), dtype=np.float64)
    for r in res.results:
        total += r["out"].astype(np.float64)
    out = (np.float32(total[0, 0]), np.float32(total[0, 1]))
    if _trace:
        return out, res
    return out
